# revision 9
# baseline (speedup 1.0000x reference)
"""Trainium2 Bass kernel for windowed Conv1d(k=3) + sigmoid gating.

Reference computation (B=16, T=960, D=1024, W=10):
  windows of size 10 are conv'd independently with per-window zero pad 1:
    cnn[t, d] = sum_{k,c} conv_w[d, c, k] * xpad[t + k, c] + conv_b[d]
    out = cnn * sigmoid(cnn @ gate_w.T + gate_b)

Strategy: pure data parallelism over the 8 NeuronCores (2 batches per
core, 192 windows = 1920 rows each). The conv uses mixed Winograd
tiling per window: two F(4,3) tiles (outputs 0-3 from xpad[0:6] and
outputs 4-7 from xpad[4:10]) plus one F(2,3) tile (outputs 8-9 from
xpad[8:12]) — 6+6+4 = 16 PE product-columns per window instead of the
30 of a direct conv (1.875x FLOP cut). F(4,3) uses interpolation
points [0, 1, -1, 2, -1/2]; since they contain the F(2,3) points
[0, 1, -1], the F(2,3) tile's transformed weights are scalar multiples
of four of the F(4,3) ones, so its columns ride along in the same
matmul streams (the scalar ratio is folded into the host-side input
transform) and every weight block loaded into the PE serves 2 groups
x 3 window-tiles. All input/weight transforms are host-side f32,
cast to bf16; m-planes accumulate in f32 PSUM; the A^T output combine
runs on ScalarE (power-of-two scaled copies) + VectorE under the
matmul stream. The gate matmul and sigmoid/multiply are unchanged
bf16; the output is DMA'd as bf16 and upcast on the host.
"""

import numpy as np
import ml_dtypes

import concourse.bacc as bacc
import concourse.bass as bass
import concourse.tile as tile
from concourse import mybir
from concourse.bass_utils import run_bass_kernel_spmd

BF16 = ml_dtypes.bfloat16

B, T, D, W = 16, 960, 1024, 10
NCORES = 8
BC = B // NCORES            # batches per core
NWIN = BC * T // W          # windows per core (192)
RC = NWIN * W               # output rows per core (1920)
NG = 4                      # column groups per core
GWIN = NWIN // NG           # windows per group (48)
GN = GWIN * W               # output columns per group (480)
NCH = D // 128              # 128-partition chunks of D (8)
AF = mybir.ActivationFunctionType

# winograd j-streams: widths per (j, ck) block of the rhs / psum planes.
# j0,j1,j2,j5 carry [A(48) | B(48) | C(48)]; j3,j4 carry [A | B].
JW = [144, 144, 144, 96, 96, 144]
JCUM = [0, 144, 288, 432, 528, 624]
CKW = 768                   # total cols per ck block (= sum(JW))
# psum plane placement: j -> (bank, col offset) inside a [128, 2, 512] tile;
# j-accumulation groups sharing a bank run strictly sequentially
# (bank0: j0, j1, j3; bank1: j2, j4, j5 in program order j0..j5).
PL = {0: (0, 0), 1: (0, 144), 3: (0, 288), 2: (1, 0), 5: (1, 144), 4: (1, 288)}

F43_PTS = [0.0, 1.0, -1.0, 2.0, -0.5]
F23_PTS = [0.0, 1.0, -1.0]


def _cook_toom(points, m, r):
    """Winograd F(m, r) matrices: y = AT @ ((G @ w) * (BT @ x))."""
    a = np.asarray(points, np.float64)
    n = m + r - 1
    AT = np.zeros((m, n))
    for j in range(n - 1):
        AT[:, j] = a[j] ** np.arange(m)
    AT[m - 1, n - 1] = 1.0
    G = np.zeros((n, r))
    for j in range(n - 1):
        fj = np.prod(np.delete(a[j] - a, j))
        G[j] = (a[j] ** np.arange(r)) / fj
    G[n - 1, r - 1] = 1.0
    BT = np.zeros((n, n))
    for j in range(n - 1):
        BT[j, :n - 1] = np.poly(np.delete(a, j))[::-1]
    BT[n - 1] = np.poly(a)[::-1]
    return AT, G, BT


AT6, G6, BT6 = _cook_toom(F43_PTS, 4, 3)
AT4, G4, BT4 = _cook_toom(F23_PTS, 2, 3)
# F23 weights are scalar multiples of F43 ones at the shared points
# (j 0,1,2 <-> c 0,1,2) and identical at infinity (j5 <-> c3).
_CJ = [0, 1, 2, 5]          # F43 j-index serving F23 column c
_CRATIO = np.array([
    np.dot(G6[j], G4[c]) / np.dot(G4[c], G4[c])
    for c, j in enumerate(_CJ)
])
# fold the m4 plane scale so its A^T coefficients become (-8, 4, -2, 1)
M4SCALE = -0.125


def _build():
    nc = bacc.Bacc("TRN2", target_bir_lowering=False, debug=False)

    # xt: [group, cc, (j, ck, cols)] winograd-transformed inputs
    xt = nc.dram_tensor("xt", [NG, 128, NCH * CKW], mybir.dt.bfloat16,
                        kind="ExternalInput")
    # cwr[dck]: [cc, ((j*NCH+ck)*128 + dd)] winograd conv lhsT blocks
    cwr = nc.dram_tensor("cwr", [NCH, 128, 6 * NCH * 128], mybir.dt.bfloat16,
                         kind="ExternalInput")
    # gwr: [dd, ((eck*NCH+dck)*128 + ee)] gate lhsT blocks (single tensor)
    gwr = nc.dram_tensor("gwr", [128, NCH * NCH * 128], mybir.dt.bfloat16,
                         kind="ExternalInput")
    cb = nc.dram_tensor("cb", [128, NCH], mybir.dt.float32, kind="ExternalInput")
    gb = nc.dram_tensor("gb", [128, NCH], mybir.dt.float32, kind="ExternalInput")
    outT = nc.dram_tensor("outT", [D, RC], mybir.dt.bfloat16,
                          kind="ExternalOutput")

    with tile.TileContext(nc) as tc:
        with (
            tc.tile_pool(name="consts", bufs=1) as consts,
            tc.tile_pool(name="work", bufs=3) as work,
            tc.tile_pool(name="cnn", bufs=2) as cnnp,
            tc.tile_pool(name="cpsum", bufs=3, space="PSUM") as cpsum,
            tc.tile_pool(name="gpsum", bufs=2, space="PSUM") as gpsum,
        ):
            cb_sb = consts.tile([128, NCH], mybir.dt.float32, tag="cb")
            gb_sb = consts.tile([128, NCH], mybir.dt.float32, tag="gb")

            xt_sb = [None] * NG
            cwr_sb = [None] * NCH

            def alloc_xg(g):
                xt_sb[g] = consts.tile([128, NCH * CKW], mybir.dt.bfloat16,
                                       name=f"xg{g}", tag=f"xg{g}")

            def alloc_cw(dck):
                cwr_sb[dck] = consts.tile([128, 6 * NCH * 128],
                                          mybir.dt.bfloat16,
                                          name=f"cw{dck}", tag=f"cw{dck}")

            def load_xg_j(g, j):
                lo, hi = NCH * JCUM[j], NCH * (JCUM[j] + JW[j])
                nc.sync.dma_start(xt_sb[g][:, lo:hi], xt[g][:, lo:hi])

            def load_cw_j(dck, j):
                lo, hi = j * NCH * 128, (j + 1) * NCH * 128
                nc.sync.dma_start(cwr_sb[dck][:, lo:hi], cwr[dck][:, lo:hi])

            # Input stream in first-use order on the single Sync HWDGE
            # queue. Each dma_start costs ~650 ns of Sync issue time, so
            # only the data the first conv j-streams block on is sliced
            # fine; everything later ships as whole tiles. xt2/xt3 and the
            # gate weights go last — they are needed tens of us after the
            # conv weight stream, which otherwise starves dck 1-7.
            for g in range(NG):
                alloc_xg(g)
            for dck in range(NCH):
                alloc_cw(dck)
            for j in range(3):
                load_xg_j(0, j)
                load_xg_j(1, j)
                load_cw_j(0, j)
            nc.sync.dma_start(cb_sb[:], cb[:])
            nc.sync.dma_start(gb_sb[:], gb[:])
            for j in range(3, 6):
                load_xg_j(0, j)
                load_xg_j(1, j)
                load_cw_j(0, j)
            for dck in range(1, NCH):
                nc.sync.dma_start(cwr_sb[dck][:], cwr[dck])
            gw_sb = consts.tile([128, NCH * NCH * 128], mybir.dt.bfloat16,
                                tag="gw")
            half = NCH * NCH * 128 // 2
            nc.sync.dma_start(gw_sb[:, :half], gwr[:, :half])
            nc.sync.dma_start(gw_sb[:, half:], gwr[:, half:])
            nc.sync.dma_start(xt_sb[2][:], xt[2])
            nc.sync.dma_start(xt_sb[3][:], xt[3])

            # Warm-up during the input-DMA bubble: throwaway matmuls flip
            # the PE HAM clock gate up just as the real stream starts.
            scr = consts.tile([128, 512], mybir.dt.bfloat16, tag="scr")
            nc.vector.memset(scr[:], 0.0)
            wps = gpsum.tile([128, GN], mybir.dt.float32, tag="gps")
            for _ in range(6):
                nc.tensor.matmul(wps[:, :480], scr[:, :128], scr[:, :480],
                                 start=True, stop=True)

            def conv_pair(ga, gb_, dck):
                psA = cpsum.tile([128, 2, 512], mybir.dt.float32, tag="cps")
                psB = cpsum.tile([128, 2, 512], mybir.dt.float32, tag="cps")
                for j in range(6):
                    bk, off = PL[j]
                    wj = JW[j]
                    for ck in range(NCH):
                        rlo = NCH * JCUM[j] + ck * wj
                        for g, ps in ((ga, psA), (gb_, psB)):
                            nc.tensor.matmul(
                                ps[:, bk, off:off + wj],
                                cwr_sb[dck][:, (j * NCH + ck) * 128:
                                            (j * NCH + ck + 1) * 128],
                                xt_sb[g][:, rlo:rlo + wj],
                                start=(ck == 0),
                                stop=(ck == NCH - 1),
                            )
                return psA, psB

            def epilogue(ps, dck):
                """A^T combine: m planes -> cnn tile [128, GN] bf16,
                columns ordered (t, win)."""
                def AB(j):
                    bk, off = PL[j]
                    return ps[:, bk, off:off + 96]

                def CC(j):
                    bk, off = PL[j]
                    return ps[:, bk, off + 96:off + 144]

                cbs = cb_sb[:, dck:dck + 1]
                # ScalarE: power-of-2 scaled copies of the m3 / m4 planes
                m3_2 = work.tile([128, 96], mybir.dt.bfloat16, tag="m3_2")
                nc.scalar.activation(m3_2[:], AB(3), AF.Copy, scale=2.0)
                m3_4 = work.tile([128, 96], mybir.dt.bfloat16, tag="m3_4")
                nc.scalar.activation(m3_4[:], m3_2[:], AF.Copy, scale=2.0)
                m3_8 = work.tile([128, 96], mybir.dt.bfloat16, tag="m3_8")
                nc.scalar.activation(m3_8[:], m3_4[:], AF.Copy, scale=2.0)
                # m4 plane is host-scaled by -1/8: coefficients (-8, 4, -2, 1)
                m4n2 = work.tile([128, 96], mybir.dt.bfloat16, tag="m4n2")
                nc.scalar.activation(m4n2[:], AB(4), AF.Copy, scale=-2.0)
                m4_4 = work.tile([128, 96], mybir.dt.bfloat16, tag="m4_4")
                nc.scalar.activation(m4_4[:], m4n2[:], AF.Copy, scale=-2.0)
                m4n8 = work.tile([128, 96], mybir.dt.bfloat16, tag="m4n8")
                nc.scalar.activation(m4n8[:], m4_4[:], AF.Copy, scale=-2.0)
                # ScalarE also stages the C-tile planes the GpSimd chain
                # needs (GpSimd cannot read PSUM).
                mc0s = work.tile([128, 48], mybir.dt.bfloat16, tag="mc0s")
                nc.scalar.activation(mc0s[:], CC(0), AF.Copy)
                mc2s = work.tile([128, 48], mybir.dt.bfloat16, tag="mc2s")
                nc.scalar.activation(mc2s[:], CC(2), AF.Copy)
                mc5s = work.tile([128, 48], mybir.dt.bfloat16, tag="mc5s")
                nc.scalar.activation(mc5s[:], CC(5), AF.Copy)
                # VectorE: chains. conv bias rides on the whole j1 plane (its
                # A^T column is all-ones for both the F(4,3) and F(2,3)
                # parts) so every output picks it up in one op.
                bk1, off1 = PL[1]
                m1b = work.tile([128, 144], mybir.dt.bfloat16, tag="m1b")
                nc.vector.tensor_scalar_add(m1b[:], ps[:, bk1, off1:off1 + 144],
                                            cbs)
                s = work.tile([128, 96], mybir.dt.bfloat16, tag="s")
                nc.vector.tensor_add(s[:], m1b[:, :96], AB(2))
                d_ = work.tile([128, 96], mybir.dt.bfloat16, tag="d_")
                nc.vector.tensor_sub(d_[:], m1b[:, :96], AB(2))

                ct = cnnp.tile([128, GN], mybir.dt.bfloat16, tag=f"cnn{dck}")
                ctv = ct[:].rearrange("q (t w) -> q t w", t=W)

                def pw(t):
                    return ctv[:, t:t + 5:4]

                t1 = work.tile([128, 96], mybir.dt.bfloat16, tag="t1")
                nc.vector.tensor_add(t1[:], AB(0), s[:])
                t2 = work.tile([128, 96], mybir.dt.bfloat16, tag="t2")
                nc.vector.tensor_add(t2[:], t1[:], AB(3))
                nc.vector.tensor_add(pw(0), t2[:], m4n8[:])
                t3 = work.tile([128, 96], mybir.dt.bfloat16, tag="t3")
                nc.vector.tensor_add(t3[:], d_[:], m3_2[:])
                nc.vector.tensor_add(pw(1), t3[:], m4_4[:])
                t4 = work.tile([128, 96], mybir.dt.bfloat16, tag="t4")
                nc.vector.tensor_add(t4[:], s[:], m3_4[:])
                nc.vector.tensor_add(pw(2), t4[:], m4n2[:])
                t5 = work.tile([128, 96], mybir.dt.bfloat16, tag="t5")
                nc.vector.tensor_add(t5[:], d_[:], AB(5))
                t6 = work.tile([128, 96], mybir.dt.bfloat16, tag="t6")
                nc.vector.tensor_add(t6[:], t5[:], m3_8[:])
                nc.vector.tensor_add(pw(3), t6[:], AB(4))
                # C tile (F(2,3)) on GpSimd (SBUF-only inputs):
                # y8 = mc0+mc1+mc2, y9 = mc1-mc2+mc3; mc1+cb is m1b's tail.
                mc1b = m1b[:, 96:144]
                t7 = work.tile([128, 48], mybir.dt.bfloat16, tag="t7")
                nc.gpsimd.tensor_add(t7[:], mc1b, mc0s[:])
                nc.gpsimd.tensor_add(ctv[:, 8], t7[:], mc2s[:])
                t8 = work.tile([128, 48], mybir.dt.bfloat16, tag="t8")
                nc.gpsimd.tensor_sub(t8[:], mc1b, mc2s[:])
                nc.gpsimd.tensor_add(ctv[:, 9], t8[:], mc5s[:])
                return ct

            outv = outT[:].rearrange("(e r) c -> r e c", r=128)

            def gate_group(g, cnnT, last=False):
                # gateT[e, r] = sigmoid(sum_d gw[d, e] * cnnT[d, r] + gb[e])
                for eh in range(2):
                    lasth = last and eh == 1
                    ot4 = work.tile([128, 4, GN], mybir.dt.bfloat16, tag="ot4")
                    for ei in range(4):
                        eck = eh * 4 + ei
                        ps2 = gpsum.tile([128, GN], mybir.dt.float32, tag="gps")
                        for dck in range(NCH):
                            nc.tensor.matmul(
                                ps2[:],
                                gw_sb[:, (eck * NCH + dck) * 128:
                                      (eck * NCH + dck + 1) * 128],
                                cnnT[dck][:],
                                start=(dck == 0),
                                stop=(dck == NCH - 1),
                            )
                        gt = work.tile([128, GN], mybir.dt.bfloat16, tag="gate")
                        chunks = ((0, GN // 2), (GN // 2, GN)) if (
                            lasth and ei == 3) else ((0, GN),)
                        for lo, hi in chunks:
                            nc.scalar.activation(gt[:, lo:hi], ps2[:, lo:hi],
                                                 AF.Sigmoid,
                                                 bias=gb_sb[:, eck:eck + 1])
                            nc.vector.tensor_mul(ot4[:, ei, lo:hi],
                                                 cnnT[eck][:, lo:hi],
                                                 gt[:, lo:hi])
                            if lasth:
                                nc.sync.dma_start(
                                    outv[:, eck:eck + 1,
                                         g * GN + lo:g * GN + hi],
                                    ot4[:, ei:ei + 1, lo:hi])
                    if not lasth:
                        nc.sync.dma_start(
                            outv[:, eh * 4:(eh + 1) * 4, g * GN:(g + 1) * GN],
                            ot4[:])

            cnn_tiles = [[None] * NCH for _ in range(NG)]
            for pair in ((0, 1), (2, 3)):
                for dck in range(NCH):
                    psA, psB = conv_pair(pair[0], pair[1], dck)
                    cnn_tiles[pair[0]][dck] = epilogue(psA, dck)
                    cnn_tiles[pair[1]][dck] = epilogue(psB, dck)
                for g in pair:
                    gate_group(g, cnn_tiles[g], last=(g == NG - 1))
    nc.compile()
    return nc


def _prep_core_input(x_shard, cw_host, gw_host, cb_host, gb_host):
    # x_shard: [BC, T, D] -> winograd-transformed [NG, 128, NCH*CKW]
    xs = x_shard.reshape(NG, GWIN, W, D)
    xp = np.zeros((NG, GWIN, 12, D), np.float32)
    xp[:, :, 1:1 + W, :] = xs
    dA = np.einsum('ji,gwic->gwjc', BT6.astype(np.float32), xp[:, :, 0:6],
                   optimize=True)
    dB = np.einsum('ji,gwic->gwjc', BT6.astype(np.float32), xp[:, :, 4:10],
                   optimize=True)
    dC = np.einsum('ji,gwic->gwjc', BT4.astype(np.float32), xp[:, :, 8:12],
                   optimize=True)
    # assemble [NG, D, (j, block)] then chunk D -> (ck, cc) with j-major
    # free layout [(j, ck, cols)]
    blk = np.empty((NG, D, CKW), np.float32)
    for j in range(6):
        o = JCUM[j]
        blk[:, :, o:o + 48] = dA[:, :, j].transpose(0, 2, 1)
        blk[:, :, o + 48:o + 96] = dB[:, :, j].transpose(0, 2, 1)
        if j in _CJ:
            c = _CJ.index(j)
            blk[:, :, o + 96:o + 144] = (np.float32(1.0 / _CRATIO[c])
                                         * dC[:, :, c]).transpose(0, 2, 1)
    # [NG, D, (j, cols48*w)] -> [NG, ck, cc, j, wj] j-major per ck? Need
    # layout [(j, ck, wj)]: currently blk is [NG, D, (j, wj)] — reorder to
    # j-major over ck: final free index = (j, ck, wj)
    xt_host = np.empty((NG, 128, NCH * CKW), np.float32)
    bv = blk.reshape(NG, NCH, 128, CKW)
    for j in range(6):
        o, wj = JCUM[j], JW[j]
        dst = xt_host[:, :, NCH * o:NCH * (o + wj)].reshape(
            NG, 128, NCH, wj)
        dst[:] = bv[:, :, :, o:o + wj].transpose(0, 2, 1, 3)
    return {"xt": xt_host.astype(BF16), "cwr": cw_host, "gwr": gw_host,
            "cb": cb_host, "gb": gb_host}


def _prep_in_maps(x, conv_w, conv_b, gate_w, gate_b):
    # conv weight transform + lhsT blocks:
    # cwr[dck][cc, (j*NCH+ck)*128 + dd] = gw_j[dck*128+dd, ck*128+cc]
    gw6 = np.einsum('jk,dck->jdc', G6.astype(np.float32), conv_w,
                    optimize=True)
    gw6[4] *= np.float32(M4SCALE)
    gt = gw6.reshape(6, NCH, 128, NCH, 128)  # [j, dck, dd, ck, cc]
    cw_host = np.ascontiguousarray(gt.transpose(1, 4, 0, 3, 2)).reshape(
        NCH, 128, 6 * NCH * 128).astype(BF16)
    # gate lhsT blocks: gwr[dd, (eck*NCH+dck)*128 + ee]
    gwt = gate_w.T.reshape(NCH, 128, NCH, 128)  # [dck, dd, eck, ee]
    gw_host = np.ascontiguousarray(gwt.transpose(1, 2, 0, 3)).reshape(
        128, NCH * NCH * 128).astype(BF16)
    cb_host = np.ascontiguousarray(conv_b.reshape(NCH, 128).T).astype(np.float32)
    gb_host = np.ascontiguousarray(gate_b.reshape(NCH, 128).T).astype(np.float32)
    return [
        _prep_core_input(x[BC * i:BC * (i + 1)], cw_host, gw_host, cb_host,
                         gb_host)
        for i in range(NCORES)
    ]


def _unshard_core(o):
    # o: [D, RC] bf16 with columns ordered (group, t, win) -> [BC, T, D] f32
    return (np.asarray(o).astype(np.float32)
            .reshape(D, NG, W, GWIN).transpose(1, 3, 2, 0)
            .reshape(NWIN, W, D).reshape(BC, T, D))


_NC_CACHE = None


def kernel(x, conv_w, conv_b, gate_w, gate_b):
    global _NC_CACHE
    x = np.asarray(x, np.float32)
    conv_w = np.asarray(conv_w, np.float32)
    conv_b = np.asarray(conv_b, np.float32)
    gate_w = np.asarray(gate_w, np.float32)
    gate_b = np.asarray(gate_b, np.float32)

    in_maps = _prep_in_maps(x, conv_w, conv_b, gate_w, gate_b)
    if _NC_CACHE is None:
        _NC_CACHE = _build()
    res = run_bass_kernel_spmd(_NC_CACHE, in_maps, core_ids=list(range(NCORES))).results

    out = np.empty((B, T, D), np.float32)
    for i in range(NCORES):
        out[BC * i:BC * (i + 1)] = _unshard_core(res[i]["outT"])
    return out


# revision 13
# speedup vs baseline: 1.1130x; 1.1130x over previous
"""Trainium2 Bass kernel for windowed Conv1d(k=3) + sigmoid gating.

Reference computation (B=16, T=960, D=1024, W=10):
  windows of size 10 are conv'd independently with per-window zero pad 1:
    cnn[t, d] = sum_{k,c} conv_w[d, c, k] * xpad[t + k, c] + conv_b[d]
    out = cnn * sigmoid(cnn @ gate_w.T + gate_b)

Strategy: pure data parallelism over the 8 NeuronCores (2 batches per
core, 192 windows = 1920 rows each). The conv uses mixed Winograd
tiling per window: two F(4,3) tiles (outputs 0-3 from xpad[0:6] and
outputs 4-7 from xpad[4:10]) plus one F(2,3) tile (outputs 8-9 from
xpad[8:12]) — 6+6+4 = 16 PE product-columns per window instead of the
30 of a direct conv (1.875x FLOP cut). F(4,3) uses interpolation
points [0, 1, -1, 2, -1/2]; since they contain the F(2,3) points
[0, 1, -1], the F(2,3) tile's transformed weights are scalar multiples
of four of the F(4,3) ones, so its columns ride along in the same
matmul streams (the scalar ratio is folded into the host-side input
transform) and every weight block loaded into the PE serves 2 groups
x 3 window-tiles. All input/weight transforms are host-side f32,
cast to bf16; m-planes accumulate in f32 PSUM; the A^T output combine
runs on ScalarE (power-of-two scaled copies) + VectorE under the
matmul stream. The gate matmul and sigmoid/multiply are unchanged
bf16; the output is DMA'd as bf16 and upcast on the host.
"""

import numpy as np
import ml_dtypes

import concourse.bacc as bacc
import concourse.bass as bass
import concourse.tile as tile
from concourse import mybir
from concourse.bass_utils import run_bass_kernel_spmd

BF16 = ml_dtypes.bfloat16

B, T, D, W = 16, 960, 1024, 10
NCORES = 8
BC = B // NCORES            # batches per core
NWIN = BC * T // W          # windows per core (192)
RC = NWIN * W               # output rows per core (1920)
NG = 4                      # column groups per core
GWIN = NWIN // NG           # windows per group (48)
GN = GWIN * W               # output columns per group (480)
NCH = D // 128              # 128-partition chunks of D (8)
AF = mybir.ActivationFunctionType

# winograd j-streams: widths per (j, ck) block of the rhs / psum planes.
# j0,j1,j2,j5 carry [A(48) | B(48) | C(48)]; j3,j4 carry [A | B].
JW = [144, 144, 144, 96, 96, 144]
JCUM = [0, 144, 288, 432, 528, 624]
CKW = 768                   # total cols per ck block (= sum(JW))
# psum plane placement: j -> (bank, col offset) inside a [128, 2, 512] tile;
# j-accumulation groups sharing a bank run strictly sequentially
# (bank0: j0, j1, j3; bank1: j2, j4, j5 in program order j0..j5).
PL = {0: (0, 0), 1: (0, 144), 3: (0, 288), 2: (1, 0), 5: (1, 144), 4: (1, 288)}

F43_PTS = [0.0, 1.0, -1.0, 2.0, -0.5]
F23_PTS = [0.0, 1.0, -1.0]


def _cook_toom(points, m, r):
    """Winograd F(m, r) matrices: y = AT @ ((G @ w) * (BT @ x))."""
    a = np.asarray(points, np.float64)
    n = m + r - 1
    AT = np.zeros((m, n))
    for j in range(n - 1):
        AT[:, j] = a[j] ** np.arange(m)
    AT[m - 1, n - 1] = 1.0
    G = np.zeros((n, r))
    for j in range(n - 1):
        fj = np.prod(np.delete(a[j] - a, j))
        G[j] = (a[j] ** np.arange(r)) / fj
    G[n - 1, r - 1] = 1.0
    BT = np.zeros((n, n))
    for j in range(n - 1):
        BT[j, :n - 1] = np.poly(np.delete(a, j))[::-1]
    BT[n - 1] = np.poly(a)[::-1]
    return AT, G, BT


AT6, G6, BT6 = _cook_toom(F43_PTS, 4, 3)
AT4, G4, BT4 = _cook_toom(F23_PTS, 2, 3)
# F23 weights are scalar multiples of F43 ones at the shared points
# (j 0,1,2 <-> c 0,1,2) and identical at infinity (j5 <-> c3).
_CJ = [0, 1, 2, 5]          # F43 j-index serving F23 column c
_CRATIO = np.array([
    np.dot(G6[j], G4[c]) / np.dot(G4[c], G4[c])
    for c, j in enumerate(_CJ)
])
# fold the m4 plane scale so its A^T coefficients become (-8, 4, -2, 1)
M4SCALE = -0.125


def _build():
    nc = bacc.Bacc("TRN2", target_bir_lowering=False, debug=False)

    # xt: [group, cc, (j, ck, cols)] winograd-transformed inputs
    xt = nc.dram_tensor("xt", [NG, 128, NCH * CKW], mybir.dt.bfloat16,
                        kind="ExternalInput")
    # cwr[dck]: [cc, ((j*NCH+ck)*128 + dd)] winograd conv lhsT blocks
    cwr = nc.dram_tensor("cwr", [NCH, 128, 6 * NCH * 128], mybir.dt.bfloat16,
                         kind="ExternalInput")
    # gwr: [dd, ((eck*NCH+dck)*128 + ee)] gate lhsT blocks (single tensor)
    gwr = nc.dram_tensor("gwr", [128, NCH * NCH * 128], mybir.dt.bfloat16,
                         kind="ExternalInput")
    cb = nc.dram_tensor("cb", [128, NCH], mybir.dt.float32, kind="ExternalInput")
    gb = nc.dram_tensor("gb", [128, NCH], mybir.dt.float32, kind="ExternalInput")
    outT = nc.dram_tensor("outT", [D, RC], mybir.dt.bfloat16,
                          kind="ExternalOutput")

    with tile.TileContext(nc) as tc:
        with (
            tc.tile_pool(name="consts", bufs=1) as consts,
            tc.tile_pool(name="work", bufs=2) as work,
            tc.tile_pool(name="cnn", bufs=2) as cnnp,
            tc.tile_pool(name="cpsum", bufs=3, space="PSUM") as cpsum,
            tc.tile_pool(name="gpsum", bufs=2, space="PSUM") as gpsum,
        ):
            cb_sb = consts.tile([128, NCH], mybir.dt.float32, tag="cb")
            gb_sb = consts.tile([128, NCH], mybir.dt.float32, tag="gb")

            xt_sb = [None] * NG
            cwr_sb = [None] * NCH

            def alloc_xg(g):
                xt_sb[g] = consts.tile([128, NCH * CKW], mybir.dt.bfloat16,
                                       name=f"xg{g}", tag=f"xg{g}")

            def alloc_cw(dck):
                cwr_sb[dck] = consts.tile([128, 6 * NCH * 128],
                                          mybir.dt.bfloat16,
                                          name=f"cw{dck}", tag=f"cw{dck}")

            def load_xg_j(g, j):
                lo, hi = NCH * JCUM[j], NCH * (JCUM[j] + JW[j])
                nc.sync.dma_start(xt_sb[g][:, lo:hi], xt[g][:, lo:hi])

            def load_cw_j(dck, j):
                lo, hi = j * NCH * 128, (j + 1) * NCH * 128
                nc.sync.dma_start(cwr_sb[dck][:, lo:hi], cwr[dck][:, lo:hi])

            # Input stream in first-use order on the single Sync HWDGE
            # queue. Each dma_start costs ~650 ns of Sync issue time, so
            # only the data the first conv j-streams block on is sliced
            # fine; everything later ships as whole tiles. xt2/xt3 and the
            # gate weights go last — they are needed tens of us after the
            # conv weight stream, which otherwise starves dck 1-7.
            for g in range(NG):
                alloc_xg(g)
            for dck in range(NCH):
                alloc_cw(dck)
            for j in range(3):
                load_xg_j(0, j)
                load_xg_j(1, j)
                load_cw_j(0, j)
            nc.sync.dma_start(cb_sb[:], cb[:])
            nc.sync.dma_start(gb_sb[:], gb[:])
            for j in range(3, 6):
                load_xg_j(0, j)
                load_xg_j(1, j)
                load_cw_j(0, j)
            for dck in range(1, NCH):
                nc.sync.dma_start(cwr_sb[dck][:], cwr[dck])
            gw_sb = consts.tile([128, NCH * NCH * 128], mybir.dt.bfloat16,
                                tag="gw")
            half = NCH * NCH * 128 // 2
            nc.sync.dma_start(gw_sb[:, :half], gwr[:, :half])
            nc.sync.dma_start(gw_sb[:, half:], gwr[:, half:])
            nc.sync.dma_start(xt_sb[2][:], xt[2])
            nc.sync.dma_start(xt_sb[3][:], xt[3])

            # Warm-up during the input-DMA bubble: throwaway matmuls flip
            # the PE HAM clock gate up just as the real stream starts.
            scr = consts.tile([128, 512], mybir.dt.bfloat16, tag="scr")
            nc.vector.memset(scr[:], 0.0)
            wps = gpsum.tile([128, GN], mybir.dt.float32, tag="gps")
            for _ in range(6):
                nc.tensor.matmul(wps[:, :480], scr[:, :128], scr[:, :480],
                                 start=True, stop=True)

            def conv_pair(ga, gb_, dck):
                psA = cpsum.tile([128, 2, 512], mybir.dt.float32, tag="cps")
                psB = cpsum.tile([128, 2, 512], mybir.dt.float32, tag="cps")
                for j in range(6):
                    bk, off = PL[j]
                    wj = JW[j]
                    for ck in range(NCH):
                        rlo = NCH * JCUM[j] + ck * wj
                        for g, ps in ((ga, psA), (gb_, psB)):
                            nc.tensor.matmul(
                                ps[:, bk, off:off + wj],
                                cwr_sb[dck][:, (j * NCH + ck) * 128:
                                            (j * NCH + ck + 1) * 128],
                                xt_sb[g][:, rlo:rlo + wj],
                                start=(ck == 0),
                                stop=(ck == NCH - 1),
                            )
                return psA, psB

            def epilogue(ps, dck, ptag):
                """A^T combine: m planes -> cnn tile [128, GN] bf16,
                columns ordered (t, win)."""
                def AB(j):
                    bk, off = PL[j]
                    return ps[:, bk, off:off + 96]

                def CC(j):
                    bk, off = PL[j]
                    return ps[:, bk, off + 96:off + 144]

                cbs = cb_sb[:, dck:dck + 1]
                # ScalarE: power-of-2 scaled copies of the m3 / m4 planes
                m3_2 = work.tile([128, 96], mybir.dt.bfloat16, tag="m3_2")
                nc.scalar.activation(m3_2[:], AB(3), AF.Copy, scale=2.0)
                m3_4 = work.tile([128, 96], mybir.dt.bfloat16, tag="m3_4")
                nc.scalar.activation(m3_4[:], m3_2[:], AF.Copy, scale=2.0)
                m3_8 = work.tile([128, 96], mybir.dt.bfloat16, tag="m3_8")
                nc.scalar.activation(m3_8[:], m3_4[:], AF.Copy, scale=2.0)
                # m4 plane is host-scaled by -1/8: coefficients (-8, 4, -2, 1)
                m4n2 = work.tile([128, 96], mybir.dt.bfloat16, tag="m4n2")
                nc.scalar.activation(m4n2[:], AB(4), AF.Copy, scale=-2.0)
                m4_4 = work.tile([128, 96], mybir.dt.bfloat16, tag="m4_4")
                nc.scalar.activation(m4_4[:], m4n2[:], AF.Copy, scale=-2.0)
                m4n8 = work.tile([128, 96], mybir.dt.bfloat16, tag="m4n8")
                nc.scalar.activation(m4n8[:], m4_4[:], AF.Copy, scale=-2.0)
                # ScalarE also stages the C-tile planes the GpSimd chain
                # needs (GpSimd cannot read PSUM).
                mc0s = work.tile([128, 48], mybir.dt.bfloat16, tag="mc0s")
                nc.scalar.activation(mc0s[:], CC(0), AF.Copy)
                mc2s = work.tile([128, 48], mybir.dt.bfloat16, tag="mc2s")
                nc.scalar.activation(mc2s[:], CC(2), AF.Copy)
                mc5s = work.tile([128, 48], mybir.dt.bfloat16, tag="mc5s")
                nc.scalar.activation(mc5s[:], CC(5), AF.Copy)
                # VectorE: chains. conv bias rides on the whole j1 plane (its
                # A^T column is all-ones for both the F(4,3) and F(2,3)
                # parts) so every output picks it up in one op.
                bk1, off1 = PL[1]
                m1b = work.tile([128, 144], mybir.dt.bfloat16, tag="m1b")
                nc.vector.tensor_scalar_add(m1b[:], ps[:, bk1, off1:off1 + 144],
                                            cbs)
                s = work.tile([128, 96], mybir.dt.bfloat16, tag="s")
                nc.vector.tensor_add(s[:], m1b[:, :96], AB(2))
                d_ = work.tile([128, 96], mybir.dt.bfloat16, tag="d_")
                nc.vector.tensor_sub(d_[:], m1b[:, :96], AB(2))

                ct = cnnp.tile([128, GN], mybir.dt.bfloat16,
                               tag=f"cnn{ptag}_{dck}")
                ctv = ct[:].rearrange("q (t w) -> q t w", t=W)

                def pw(t):
                    return ctv[:, t:t + 5:4]

                t1 = work.tile([128, 96], mybir.dt.bfloat16, tag="t1")
                nc.vector.tensor_add(t1[:], AB(0), s[:])
                t2 = work.tile([128, 96], mybir.dt.bfloat16, tag="t2")
                nc.vector.tensor_add(t2[:], t1[:], AB(3))
                nc.vector.tensor_add(pw(0), t2[:], m4n8[:])
                t3 = work.tile([128, 96], mybir.dt.bfloat16, tag="t3")
                nc.vector.tensor_add(t3[:], d_[:], m3_2[:])
                nc.vector.tensor_add(pw(1), t3[:], m4_4[:])
                t4 = work.tile([128, 96], mybir.dt.bfloat16, tag="t4")
                nc.vector.tensor_add(t4[:], s[:], m3_4[:])
                nc.vector.tensor_add(pw(2), t4[:], m4n2[:])
                t5 = work.tile([128, 96], mybir.dt.bfloat16, tag="t5")
                nc.vector.tensor_add(t5[:], d_[:], AB(5))
                t6 = work.tile([128, 96], mybir.dt.bfloat16, tag="t6")
                nc.vector.tensor_add(t6[:], t5[:], m3_8[:])
                nc.vector.tensor_add(pw(3), t6[:], AB(4))
                # C tile (F(2,3)) on GpSimd (SBUF-only inputs):
                # y8 = mc0+mc1+mc2, y9 = mc1-mc2+mc3; mc1+cb is m1b's tail.
                mc1b = m1b[:, 96:144]
                t7 = work.tile([128, 48], mybir.dt.bfloat16, tag="t7")
                nc.gpsimd.tensor_add(t7[:], mc1b, mc0s[:])
                nc.gpsimd.tensor_add(ctv[:, 8], t7[:], mc2s[:])
                t8 = work.tile([128, 48], mybir.dt.bfloat16, tag="t8")
                nc.gpsimd.tensor_sub(t8[:], mc1b, mc2s[:])
                nc.gpsimd.tensor_add(ctv[:, 9], t8[:], mc5s[:])
                return ct

            outv = outT[:].rearrange("(e r) c -> r e c", r=128)

            def gate_quad(g, eh, cnnT, lasth=False):
                # gateT[e, r] = sigmoid(sum_d gw[d, e] * cnnT[d, r] + gb[e])
                # for 4 eck chunks; one batched output DMA per quad.
                ot4 = work.tile([128, 4, GN], mybir.dt.bfloat16, tag="ot4",
                                bufs=2)
                for ei in range(4):
                    eck = eh * 4 + ei
                    ps2 = gpsum.tile([128, GN], mybir.dt.float32, tag="gps")
                    for dck in range(NCH):
                        nc.tensor.matmul(
                            ps2[:],
                            gw_sb[:, (eck * NCH + dck) * 128:
                                  (eck * NCH + dck + 1) * 128],
                            cnnT[dck][:],
                            start=(dck == 0),
                            stop=(dck == NCH - 1),
                        )
                    gt = work.tile([128, GN], mybir.dt.bfloat16, tag="gate",
                                   bufs=2)
                    chunks = ((0, GN // 2), (GN // 2, GN)) if (
                        lasth and ei == 3) else ((0, GN),)
                    for lo, hi in chunks:
                        nc.scalar.activation(gt[:, lo:hi], ps2[:, lo:hi],
                                             AF.Sigmoid,
                                             bias=gb_sb[:, eck:eck + 1])
                        nc.vector.tensor_mul(ot4[:, ei, lo:hi],
                                             cnnT[eck][:, lo:hi],
                                             gt[:, lo:hi])
                        if lasth:
                            nc.sync.dma_start(
                                outv[:, eck:eck + 1,
                                     g * GN + lo:g * GN + hi],
                                ot4[:, ei:ei + 1, lo:hi])
                if not lasth:
                    nc.sync.dma_start(
                        outv[:, eh * 4:(eh + 1) * 4, g * GN:(g + 1) * GN],
                        ot4[:])

            cnn_tiles = [[None] * NCH for _ in range(NG)]
            # pair 0 convs
            for dck in range(NCH):
                psA, psB = conv_pair(0, 1, dck)
                cnn_tiles[0][dck] = epilogue(psA, dck, 0)
                cnn_tiles[1][dck] = epilogue(psB, dck, 0)
            # pair 0 gates interleaved with pair 1 convs: the epilogue's
            # VectorE/ScalarE work spreads into the gate matmul stretches,
            # where those engines would otherwise idle.
            for dck in range(NCH):
                psA, psB = conv_pair(2, 3, dck)
                cnn_tiles[2][dck] = epilogue(psA, dck, 1)
                cnn_tiles[3][dck] = epilogue(psB, dck, 1)
                if dck % 2 == 1:
                    k = dck // 2
                    gate_quad(k // 2, k % 2, cnn_tiles[k // 2])
            # pair 1 gates
            for g in (2, 3):
                for eh in range(2):
                    gate_quad(g, eh, cnn_tiles[g],
                              lasth=(g == 3 and eh == 1))
    nc.compile()
    return nc


def _prep_core_input(x_shard, cw_host, gw_host, cb_host, gb_host):
    # x_shard: [BC, T, D] -> winograd-transformed [NG, 128, NCH*CKW]
    xs = x_shard.reshape(NG, GWIN, W, D)
    xp = np.zeros((NG, GWIN, 12, D), np.float32)
    xp[:, :, 1:1 + W, :] = xs
    dA = np.einsum('ji,gwic->gwjc', BT6.astype(np.float32), xp[:, :, 0:6],
                   optimize=True)
    dB = np.einsum('ji,gwic->gwjc', BT6.astype(np.float32), xp[:, :, 4:10],
                   optimize=True)
    dC = np.einsum('ji,gwic->gwjc', BT4.astype(np.float32), xp[:, :, 8:12],
                   optimize=True)
    # assemble [NG, D, (j, block)] then chunk D -> (ck, cc) with j-major
    # free layout [(j, ck, cols)]
    blk = np.empty((NG, D, CKW), np.float32)
    for j in range(6):
        o = JCUM[j]
        blk[:, :, o:o + 48] = dA[:, :, j].transpose(0, 2, 1)
        blk[:, :, o + 48:o + 96] = dB[:, :, j].transpose(0, 2, 1)
        if j in _CJ:
            c = _CJ.index(j)
            blk[:, :, o + 96:o + 144] = (np.float32(1.0 / _CRATIO[c])
                                         * dC[:, :, c]).transpose(0, 2, 1)
    # [NG, D, (j, cols48*w)] -> [NG, ck, cc, j, wj] j-major per ck? Need
    # layout [(j, ck, wj)]: currently blk is [NG, D, (j, wj)] — reorder to
    # j-major over ck: final free index = (j, ck, wj)
    xt_host = np.empty((NG, 128, NCH * CKW), np.float32)
    bv = blk.reshape(NG, NCH, 128, CKW)
    for j in range(6):
        o, wj = JCUM[j], JW[j]
        dst = xt_host[:, :, NCH * o:NCH * (o + wj)].reshape(
            NG, 128, NCH, wj)
        dst[:] = bv[:, :, :, o:o + wj].transpose(0, 2, 1, 3)
    return {"xt": xt_host.astype(BF16), "cwr": cw_host, "gwr": gw_host,
            "cb": cb_host, "gb": gb_host}


def _prep_in_maps(x, conv_w, conv_b, gate_w, gate_b):
    # conv weight transform + lhsT blocks:
    # cwr[dck][cc, (j*NCH+ck)*128 + dd] = gw_j[dck*128+dd, ck*128+cc]
    gw6 = np.einsum('jk,dck->jdc', G6.astype(np.float32), conv_w,
                    optimize=True)
    gw6[4] *= np.float32(M4SCALE)
    gt = gw6.reshape(6, NCH, 128, NCH, 128)  # [j, dck, dd, ck, cc]
    cw_host = np.ascontiguousarray(gt.transpose(1, 4, 0, 3, 2)).reshape(
        NCH, 128, 6 * NCH * 128).astype(BF16)
    # gate lhsT blocks: gwr[dd, (eck*NCH+dck)*128 + ee]
    gwt = gate_w.T.reshape(NCH, 128, NCH, 128)  # [dck, dd, eck, ee]
    gw_host = np.ascontiguousarray(gwt.transpose(1, 2, 0, 3)).reshape(
        128, NCH * NCH * 128).astype(BF16)
    cb_host = np.ascontiguousarray(conv_b.reshape(NCH, 128).T).astype(np.float32)
    gb_host = np.ascontiguousarray(gate_b.reshape(NCH, 128).T).astype(np.float32)
    return [
        _prep_core_input(x[BC * i:BC * (i + 1)], cw_host, gw_host, cb_host,
                         gb_host)
        for i in range(NCORES)
    ]


def _unshard_core(o):
    # o: [D, RC] bf16 with columns ordered (group, t, win) -> [BC, T, D] f32
    return (np.asarray(o).astype(np.float32)
            .reshape(D, NG, W, GWIN).transpose(1, 3, 2, 0)
            .reshape(NWIN, W, D).reshape(BC, T, D))


_NC_CACHE = None


def kernel(x, conv_w, conv_b, gate_w, gate_b):
    global _NC_CACHE
    x = np.asarray(x, np.float32)
    conv_w = np.asarray(conv_w, np.float32)
    conv_b = np.asarray(conv_b, np.float32)
    gate_w = np.asarray(gate_w, np.float32)
    gate_b = np.asarray(gate_b, np.float32)

    in_maps = _prep_in_maps(x, conv_w, conv_b, gate_w, gate_b)
    if _NC_CACHE is None:
        _NC_CACHE = _build()
    res = run_bass_kernel_spmd(_NC_CACHE, in_maps, core_ids=list(range(NCORES))).results

    out = np.empty((B, T, D), np.float32)
    for i in range(NCORES):
        out[BC * i:BC * (i + 1)] = _unshard_core(res[i]["outT"])
    return out


# revision 18
# speedup vs baseline: 1.1187x; 1.0052x over previous
"""Trainium2 Bass kernel for windowed Conv1d(k=3) + sigmoid gating.

Reference computation (B=16, T=960, D=1024, W=10):
  windows of size 10 are conv'd independently with per-window zero pad 1:
    cnn[t, d] = sum_{k,c} conv_w[d, c, k] * xpad[t + k, c] + conv_b[d]
    out = cnn * sigmoid(cnn @ gate_w.T + gate_b)

Strategy: pure data parallelism over the 8 NeuronCores (2 batches per
core, 192 windows = 1920 rows each). The conv uses mixed Winograd
tiling per window: two F(4,3) tiles (outputs 0-3 from xpad[0:6] and
outputs 4-7 from xpad[4:10]) plus one F(2,3) tile (outputs 8-9 from
xpad[8:12]) — 6+6+4 = 16 PE product-columns per window instead of the
30 of a direct conv (1.875x FLOP cut). F(4,3) uses interpolation
points [0, 1, -1, 2, -1/2]; since they contain the F(2,3) points
[0, 1, -1], the F(2,3) tile's transformed weights are scalar multiples
of four of the F(4,3) ones, so its columns ride along in the same
matmul streams (the scalar ratio is folded into the host-side input
transform) and every weight block loaded into the PE serves 2 groups
x 3 window-tiles. All input/weight transforms are host-side f32,
cast to bf16; m-planes accumulate in f32 PSUM; the A^T output combine
runs on ScalarE (power-of-two scaled copies) + VectorE under the
matmul stream. The gate matmul and sigmoid/multiply are unchanged
bf16; the output is DMA'd as bf16 and upcast on the host.
"""

import numpy as np
import ml_dtypes

import concourse.bacc as bacc
import concourse.bass as bass
import concourse.tile as tile
from concourse import mybir
from concourse.bass_utils import run_bass_kernel_spmd

BF16 = ml_dtypes.bfloat16

B, T, D, W = 16, 960, 1024, 10
NCORES = 8
BC = B // NCORES            # batches per core
NWIN = BC * T // W          # windows per core (192)
RC = NWIN * W               # output rows per core (1920)
NG = 4                      # column groups per core
GWIN = NWIN // NG           # windows per group (48)
GN = GWIN * W               # output columns per group (480)
NCH = D // 128              # 128-partition chunks of D (8)
AF = mybir.ActivationFunctionType

# winograd j-streams: widths per (j, ck) block of the rhs / psum planes.
# j0,j1,j2,j5 carry [A(48) | B(48) | C(48)]; j3,j4 carry [A | B].
JW = [144, 144, 144, 96, 96, 144]
JCUM = [0, 144, 288, 432, 528, 624]
CKW = 768                   # total cols per ck block (= sum(JW))
# psum plane placement: j -> (bank, col offset) inside a [128, 2, 512] tile;
# j-accumulation groups sharing a bank run strictly sequentially
# (bank0: j0, j1, j3; bank1: j2, j4, j5 in program order j0..j5).
PL = {0: (0, 0), 1: (0, 144), 3: (0, 288), 2: (1, 0), 5: (1, 144), 4: (1, 288)}

F43_PTS = [0.0, 1.0, -1.0, 2.0, -0.5]
F23_PTS = [0.0, 1.0, -1.0]


def _cook_toom(points, m, r):
    """Winograd F(m, r) matrices: y = AT @ ((G @ w) * (BT @ x))."""
    a = np.asarray(points, np.float64)
    n = m + r - 1
    AT = np.zeros((m, n))
    for j in range(n - 1):
        AT[:, j] = a[j] ** np.arange(m)
    AT[m - 1, n - 1] = 1.0
    G = np.zeros((n, r))
    for j in range(n - 1):
        fj = np.prod(np.delete(a[j] - a, j))
        G[j] = (a[j] ** np.arange(r)) / fj
    G[n - 1, r - 1] = 1.0
    BT = np.zeros((n, n))
    for j in range(n - 1):
        BT[j, :n - 1] = np.poly(np.delete(a, j))[::-1]
    BT[n - 1] = np.poly(a)[::-1]
    return AT, G, BT


AT6, G6, BT6 = _cook_toom(F43_PTS, 4, 3)
AT4, G4, BT4 = _cook_toom(F23_PTS, 2, 3)
# F23 weights are scalar multiples of F43 ones at the shared points
# (j 0,1,2 <-> c 0,1,2) and identical at infinity (j5 <-> c3).
_CJ = [0, 1, 2, 5]          # F43 j-index serving F23 column c
_CRATIO = np.array([
    np.dot(G6[j], G4[c]) / np.dot(G4[c], G4[c])
    for c, j in enumerate(_CJ)
])
# fold the m4 plane scale so its A^T coefficients become (-8, 4, -2, 1)
M4SCALE = -0.125


def _build():
    nc = bacc.Bacc("TRN2", target_bir_lowering=False, debug=False)

    # xt: [group, cc, (j, ck, cols)] winograd-transformed inputs
    xt = nc.dram_tensor("xt", [NG, 128, NCH * CKW], mybir.dt.bfloat16,
                        kind="ExternalInput")
    # cwr[dck]: [cc, ((j*NCH+ck)*128 + dd)] winograd conv lhsT blocks
    cwr = nc.dram_tensor("cwr", [NCH, 128, 6 * NCH * 128], mybir.dt.bfloat16,
                         kind="ExternalInput")
    # gwr: [dd, ((eck*NCH+dck)*128 + ee)] gate lhsT blocks (single tensor)
    gwr = nc.dram_tensor("gwr", [128, NCH * NCH * 128], mybir.dt.bfloat16,
                         kind="ExternalInput")
    cbg = nc.dram_tensor("cbg", [128, 2 * NCH], mybir.dt.float32,
                         kind="ExternalInput")
    outT = nc.dram_tensor("outT", [D, RC], mybir.dt.bfloat16,
                          kind="ExternalOutput")

    with tile.TileContext(nc) as tc:
        with (
            tc.tile_pool(name="consts", bufs=1) as consts,
            tc.tile_pool(name="work", bufs=2) as work,
            tc.tile_pool(name="cnn", bufs=2) as cnnp,
            tc.tile_pool(name="cpsum", bufs=3, space="PSUM") as cpsum,
            tc.tile_pool(name="gpsum", bufs=2, space="PSUM") as gpsum,
        ):
            cbg_sb = consts.tile([128, 2 * NCH], mybir.dt.float32, tag="cbg")
            cb_sb = cbg_sb[:, :NCH]
            gb_sb = cbg_sb[:, NCH:]

            xt_sb = [None] * NG
            cwr_sb = [None] * NCH

            def alloc_xg(g):
                xt_sb[g] = consts.tile([128, NCH * CKW], mybir.dt.bfloat16,
                                       name=f"xg{g}", tag=f"xg{g}")

            def alloc_cw(dck):
                cwr_sb[dck] = consts.tile([128, 6 * NCH * 128],
                                          mybir.dt.bfloat16,
                                          name=f"cw{dck}", tag=f"cw{dck}")

            def load_xg_jp(g, j0, j1):
                lo, hi = NCH * JCUM[j0], NCH * (JCUM[j1] + JW[j1])
                nc.sync.dma_start(xt_sb[g][:, lo:hi], xt[g][:, lo:hi])

            def load_cw_jp(dck, j0, j1):
                lo, hi = j0 * NCH * 128, (j1 + 1) * NCH * 128
                nc.sync.dma_start(cwr_sb[dck][:, lo:hi], cwr[dck][:, lo:hi])

            # Input stream in first-use order on the single Sync HWDGE
            # queue. Each dma_start costs ~650 ns of Sync issue time, so
            # only the data the first conv j-streams block on is sliced
            # (in j-pairs); everything later ships whole. xt2/xt3 precede
            # the gate weights: they are needed at the pair-1 convs, the
            # gate weights only at the first interleaved gate quad.
            for g in range(NG):
                alloc_xg(g)
            for dck in range(NCH):
                alloc_cw(dck)
            for j0 in (0, 2, 4):
                load_xg_jp(0, j0, j0 + 1)
                load_xg_jp(1, j0, j0 + 1)
                load_cw_jp(0, j0, j0 + 1)
                if j0 == 0:
                    nc.sync.dma_start(cbg_sb[:], cbg[:])
            for dck in range(1, NCH):
                nc.sync.dma_start(cwr_sb[dck][:], cwr[dck])
            nc.sync.dma_start(xt_sb[2][:], xt[2])
            nc.sync.dma_start(xt_sb[3][:], xt[3])
            gw_sb = consts.tile([128, NCH * NCH * 128], mybir.dt.bfloat16,
                                tag="gw")
            half = NCH * NCH * 128 // 2
            nc.sync.dma_start(gw_sb[:, :half], gwr[:, :half])
            nc.sync.dma_start(gw_sb[:, half:], gwr[:, half:])

            # Warm-up during the input-DMA bubble: throwaway matmuls flip
            # the PE HAM clock gate up just as the real stream starts.
            scr = consts.tile([128, 512], mybir.dt.bfloat16, tag="scr")
            nc.vector.memset(scr[:], 0.0)
            wps = gpsum.tile([128, GN], mybir.dt.float32, tag="gps")
            for _ in range(12):
                nc.tensor.matmul(wps[:, :480], scr[:, :128], scr[:, :480],
                                 start=True, stop=True)

            def conv_pair(ga, gb_, dck):
                psA = cpsum.tile([128, 2, 512], mybir.dt.float32, tag="cps")
                psB = cpsum.tile([128, 2, 512], mybir.dt.float32, tag="cps")
                for j in range(6):
                    bk, off = PL[j]
                    wj = JW[j]
                    for ck in range(NCH):
                        rlo = NCH * JCUM[j] + ck * wj
                        for g, ps in ((ga, psA), (gb_, psB)):
                            nc.tensor.matmul(
                                ps[:, bk, off:off + wj],
                                cwr_sb[dck][:, (j * NCH + ck) * 128:
                                            (j * NCH + ck + 1) * 128],
                                xt_sb[g][:, rlo:rlo + wj],
                                start=(ck == 0),
                                stop=(ck == NCH - 1),
                            )
                return psA, psB

            def epilogue(ps, dck, ptag):
                """A^T combine: m planes -> cnn tile [128, GN] bf16,
                columns ordered (t, win)."""
                def AB(j):
                    bk, off = PL[j]
                    return ps[:, bk, off:off + 96]

                def CC(j):
                    bk, off = PL[j]
                    return ps[:, bk, off + 96:off + 144]

                cbs = cb_sb[:, dck:dck + 1]
                # ScalarE: power-of-2 scaled copies of the m3 / m4 planes
                m3_2 = work.tile([128, 96], mybir.dt.bfloat16, tag="m3_2")
                nc.scalar.activation(m3_2[:], AB(3), AF.Copy, scale=2.0)
                m3_4 = work.tile([128, 96], mybir.dt.bfloat16, tag="m3_4")
                nc.scalar.activation(m3_4[:], m3_2[:], AF.Copy, scale=2.0)
                m3_8 = work.tile([128, 96], mybir.dt.bfloat16, tag="m3_8")
                nc.scalar.activation(m3_8[:], m3_4[:], AF.Copy, scale=2.0)
                # m4 plane is host-scaled by -1/8: coefficients (-8, 4, -2, 1)
                m4n2 = work.tile([128, 96], mybir.dt.bfloat16, tag="m4n2")
                nc.scalar.activation(m4n2[:], AB(4), AF.Copy, scale=-2.0)
                m4_4 = work.tile([128, 96], mybir.dt.bfloat16, tag="m4_4")
                nc.scalar.activation(m4_4[:], m4n2[:], AF.Copy, scale=-2.0)
                m4n8 = work.tile([128, 96], mybir.dt.bfloat16, tag="m4n8")
                nc.scalar.activation(m4n8[:], m4_4[:], AF.Copy, scale=-2.0)
                # ScalarE also stages the C-tile planes the GpSimd chain
                # needs (GpSimd cannot read PSUM).
                mc0s = work.tile([128, 48], mybir.dt.bfloat16, tag="mc0s")
                nc.scalar.activation(mc0s[:], CC(0), AF.Copy)
                mc2s = work.tile([128, 48], mybir.dt.bfloat16, tag="mc2s")
                nc.scalar.activation(mc2s[:], CC(2), AF.Copy)
                mc5s = work.tile([128, 48], mybir.dt.bfloat16, tag="mc5s")
                nc.scalar.activation(mc5s[:], CC(5), AF.Copy)
                # VectorE: chains. conv bias rides on the whole j1 plane (its
                # A^T column is all-ones for both the F(4,3) and F(2,3)
                # parts) so every output picks it up in one op.
                bk1, off1 = PL[1]
                m1b = work.tile([128, 144], mybir.dt.bfloat16, tag="m1b")
                nc.vector.tensor_scalar_add(m1b[:], ps[:, bk1, off1:off1 + 144],
                                            cbs)
                s = work.tile([128, 96], mybir.dt.bfloat16, tag="s")
                nc.vector.tensor_add(s[:], m1b[:, :96], AB(2))
                d_ = work.tile([128, 96], mybir.dt.bfloat16, tag="d_")
                nc.vector.tensor_sub(d_[:], m1b[:, :96], AB(2))

                ct = cnnp.tile([128, GN], mybir.dt.bfloat16,
                               tag=f"cnn{ptag}_{dck}")
                ctv = ct[:].rearrange("q (t w) -> q t w", t=W)

                def pw(t):
                    return ctv[:, t:t + 5:4]

                t1 = work.tile([128, 96], mybir.dt.bfloat16, tag="t1")
                nc.vector.tensor_add(t1[:], AB(0), s[:])
                t2 = work.tile([128, 96], mybir.dt.bfloat16, tag="t2")
                nc.vector.tensor_add(t2[:], t1[:], AB(3))
                nc.vector.tensor_add(pw(0), t2[:], m4n8[:])
                t3 = work.tile([128, 96], mybir.dt.bfloat16, tag="t3")
                nc.vector.tensor_add(t3[:], d_[:], m3_2[:])
                nc.vector.tensor_add(pw(1), t3[:], m4_4[:])
                t4 = work.tile([128, 96], mybir.dt.bfloat16, tag="t4")
                nc.vector.tensor_add(t4[:], s[:], m3_4[:])
                nc.vector.tensor_add(pw(2), t4[:], m4n2[:])
                t5 = work.tile([128, 96], mybir.dt.bfloat16, tag="t5")
                nc.vector.tensor_add(t5[:], d_[:], AB(5))
                t6 = work.tile([128, 96], mybir.dt.bfloat16, tag="t6")
                nc.vector.tensor_add(t6[:], t5[:], m3_8[:])
                nc.vector.tensor_add(pw(3), t6[:], AB(4))
                # C tile (F(2,3)) on GpSimd (SBUF-only inputs):
                # y8 = mc0+mc1+mc2, y9 = mc1-mc2+mc3; mc1+cb is m1b's tail.
                mc1b = m1b[:, 96:144]
                t7 = work.tile([128, 48], mybir.dt.bfloat16, tag="t7")
                nc.gpsimd.tensor_add(t7[:], mc1b, mc0s[:])
                nc.gpsimd.tensor_add(ctv[:, 8], t7[:], mc2s[:])
                t8 = work.tile([128, 48], mybir.dt.bfloat16, tag="t8")
                nc.gpsimd.tensor_sub(t8[:], mc1b, mc2s[:])
                nc.gpsimd.tensor_add(ctv[:, 9], t8[:], mc5s[:])
                return ct

            outv = outT[:].rearrange("(e r) c -> r e c", r=128)

            def gate_quad(g, eh, cnnT, lasth=False):
                # gateT[e, r] = sigmoid(sum_d gw[d, e] * cnnT[d, r] + gb[e])
                # for 4 eck chunks; one batched output DMA per quad.
                ot4 = work.tile([128, 4, GN], mybir.dt.bfloat16, tag="ot4",
                                bufs=2)
                for ei in range(4):
                    eck = eh * 4 + ei
                    ps2 = gpsum.tile([128, GN], mybir.dt.float32, tag="gps")
                    for dck in range(NCH):
                        nc.tensor.matmul(
                            ps2[:],
                            gw_sb[:, (eck * NCH + dck) * 128:
                                  (eck * NCH + dck + 1) * 128],
                            cnnT[dck][:],
                            start=(dck == 0),
                            stop=(dck == NCH - 1),
                        )
                    gt = work.tile([128, GN], mybir.dt.bfloat16, tag="gate",
                                   bufs=2)
                    chunks = ((0, GN // 2), (GN // 2, GN)) if (
                        lasth and ei == 3) else ((0, GN),)
                    for lo, hi in chunks:
                        nc.scalar.activation(gt[:, lo:hi], ps2[:, lo:hi],
                                             AF.Sigmoid,
                                             bias=gb_sb[:, eck:eck + 1])
                        nc.vector.tensor_mul(ot4[:, ei, lo:hi],
                                             cnnT[eck][:, lo:hi],
                                             gt[:, lo:hi])
                        if lasth:
                            nc.sync.dma_start(
                                outv[:, eck:eck + 1,
                                     g * GN + lo:g * GN + hi],
                                ot4[:, ei:ei + 1, lo:hi])
                if not lasth:
                    nc.sync.dma_start(
                        outv[:, eh * 4:(eh + 1) * 4, g * GN:(g + 1) * GN],
                        ot4[:])

            cnn_tiles = [[None] * NCH for _ in range(NG)]
            # pair 0 convs
            for dck in range(NCH):
                psA, psB = conv_pair(0, 1, dck)
                cnn_tiles[0][dck] = epilogue(psA, dck, 0)
                cnn_tiles[1][dck] = epilogue(psB, dck, 0)
            # pair 0 gates interleaved with pair 1 convs: the epilogue's
            # VectorE/ScalarE work spreads into the gate matmul stretches,
            # where those engines would otherwise idle.
            for dck in range(NCH):
                psA, psB = conv_pair(2, 3, dck)
                cnn_tiles[2][dck] = epilogue(psA, dck, 1)
                cnn_tiles[3][dck] = epilogue(psB, dck, 1)
                if dck % 2 == 1:
                    k = dck // 2
                    gate_quad(k // 2, k % 2, cnn_tiles[k // 2])
            # pair 1 gates
            for g in (2, 3):
                for eh in range(2):
                    gate_quad(g, eh, cnn_tiles[g],
                              lasth=(g == 3 and eh == 1))
    nc.compile()
    return nc


def _prep_core_input(x_shard, cw_host, gw_host, cbg_host):
    # x_shard: [BC, T, D] -> winograd-transformed [NG, 128, NCH*CKW]
    xs = x_shard.reshape(NG, GWIN, W, D)
    xp = np.zeros((NG, GWIN, 12, D), np.float32)
    xp[:, :, 1:1 + W, :] = xs
    dA = np.einsum('ji,gwic->gwjc', BT6.astype(np.float32), xp[:, :, 0:6],
                   optimize=True)
    dB = np.einsum('ji,gwic->gwjc', BT6.astype(np.float32), xp[:, :, 4:10],
                   optimize=True)
    dC = np.einsum('ji,gwic->gwjc', BT4.astype(np.float32), xp[:, :, 8:12],
                   optimize=True)
    # assemble [NG, D, (j, block)] then chunk D -> (ck, cc) with j-major
    # free layout [(j, ck, cols)]
    blk = np.empty((NG, D, CKW), np.float32)
    for j in range(6):
        o = JCUM[j]
        blk[:, :, o:o + 48] = dA[:, :, j].transpose(0, 2, 1)
        blk[:, :, o + 48:o + 96] = dB[:, :, j].transpose(0, 2, 1)
        if j in _CJ:
            c = _CJ.index(j)
            blk[:, :, o + 96:o + 144] = (np.float32(1.0 / _CRATIO[c])
                                         * dC[:, :, c]).transpose(0, 2, 1)
    # [NG, D, (j, cols48*w)] -> [NG, ck, cc, j, wj] j-major per ck? Need
    # layout [(j, ck, wj)]: currently blk is [NG, D, (j, wj)] — reorder to
    # j-major over ck: final free index = (j, ck, wj)
    xt_host = np.empty((NG, 128, NCH * CKW), np.float32)
    bv = blk.reshape(NG, NCH, 128, CKW)
    for j in range(6):
        o, wj = JCUM[j], JW[j]
        dst = xt_host[:, :, NCH * o:NCH * (o + wj)].reshape(
            NG, 128, NCH, wj)
        dst[:] = bv[:, :, :, o:o + wj].transpose(0, 2, 1, 3)
    return {"xt": xt_host.astype(BF16), "cwr": cw_host, "gwr": gw_host,
            "cbg": cbg_host}


def _prep_in_maps(x, conv_w, conv_b, gate_w, gate_b):
    # conv weight transform + lhsT blocks:
    # cwr[dck][cc, (j*NCH+ck)*128 + dd] = gw_j[dck*128+dd, ck*128+cc]
    gw6 = np.einsum('jk,dck->jdc', G6.astype(np.float32), conv_w,
                    optimize=True)
    gw6[4] *= np.float32(M4SCALE)
    gt = gw6.reshape(6, NCH, 128, NCH, 128)  # [j, dck, dd, ck, cc]
    cw_host = np.ascontiguousarray(gt.transpose(1, 4, 0, 3, 2)).reshape(
        NCH, 128, 6 * NCH * 128).astype(BF16)
    # gate lhsT blocks: gwr[dd, (eck*NCH+dck)*128 + ee]
    gwt = gate_w.T.reshape(NCH, 128, NCH, 128)  # [dck, dd, eck, ee]
    gw_host = np.ascontiguousarray(gwt.transpose(1, 2, 0, 3)).reshape(
        128, NCH * NCH * 128).astype(BF16)
    cbg_host = np.ascontiguousarray(np.concatenate(
        [conv_b.reshape(NCH, 128).T, gate_b.reshape(NCH, 128).T],
        axis=1)).astype(np.float32)
    return [
        _prep_core_input(x[BC * i:BC * (i + 1)], cw_host, gw_host, cbg_host)
        for i in range(NCORES)
    ]


def _unshard_core(o):
    # o: [D, RC] bf16 with columns ordered (group, t, win) -> [BC, T, D] f32
    return (np.asarray(o).astype(np.float32)
            .reshape(D, NG, W, GWIN).transpose(1, 3, 2, 0)
            .reshape(NWIN, W, D).reshape(BC, T, D))


_NC_CACHE = None


def kernel(x, conv_w, conv_b, gate_w, gate_b):
    global _NC_CACHE
    x = np.asarray(x, np.float32)
    conv_w = np.asarray(conv_w, np.float32)
    conv_b = np.asarray(conv_b, np.float32)
    gate_w = np.asarray(gate_w, np.float32)
    gate_b = np.asarray(gate_b, np.float32)

    in_maps = _prep_in_maps(x, conv_w, conv_b, gate_w, gate_b)
    if _NC_CACHE is None:
        _NC_CACHE = _build()
    res = run_bass_kernel_spmd(_NC_CACHE, in_maps, core_ids=list(range(NCORES))).results

    out = np.empty((B, T, D), np.float32)
    for i in range(NCORES):
        out[BC * i:BC * (i + 1)] = _unshard_core(res[i]["outT"])
    return out


# revision 21
# speedup vs baseline: 1.1252x; 1.0058x over previous
"""Trainium2 Bass kernel for windowed Conv1d(k=3) + sigmoid gating.

Reference computation (B=16, T=960, D=1024, W=10):
  windows of size 10 are conv'd independently with per-window zero pad 1:
    cnn[t, d] = sum_{k,c} conv_w[d, c, k] * xpad[t + k, c] + conv_b[d]
    out = cnn * sigmoid(cnn @ gate_w.T + gate_b)

Strategy: pure data parallelism over the 8 NeuronCores (2 batches per
core, 192 windows = 1920 rows each). The conv uses mixed Winograd
tiling per window: two F(4,3) tiles (outputs 0-3 from xpad[0:6] and
outputs 4-7 from xpad[4:10]) plus one F(2,3) tile (outputs 8-9 from
xpad[8:12]) — 6+6+4 = 16 PE product-columns per window instead of the
30 of a direct conv (1.875x FLOP cut). F(4,3) uses interpolation
points [0, 1, -1, 2, -1/2]; since they contain the F(2,3) points
[0, 1, -1], the F(2,3) tile's transformed weights are scalar multiples
of four of the F(4,3) ones, so its columns ride along in the same
matmul streams (the scalar ratio is folded into the host-side input
transform) and every weight block loaded into the PE serves 2 groups
x 3 window-tiles. All input/weight transforms are host-side f32,
cast to bf16; m-planes accumulate in f32 PSUM; the A^T output combine
runs on ScalarE (power-of-two scaled copies) + VectorE under the
matmul stream. The gate matmul and sigmoid/multiply are unchanged
bf16; the output is DMA'd as bf16 and upcast on the host.
"""

import numpy as np
import ml_dtypes

import concourse.bacc as bacc
import concourse.bass as bass
import concourse.tile as tile
from concourse import mybir
from concourse.bass_utils import run_bass_kernel_spmd

BF16 = ml_dtypes.bfloat16

B, T, D, W = 16, 960, 1024, 10
NCORES = 8
BC = B // NCORES            # batches per core
NWIN = BC * T // W          # windows per core (192)
RC = NWIN * W               # output rows per core (1920)
NG = 4                      # column groups per core
GWIN = NWIN // NG           # windows per group (48)
GN = GWIN * W               # output columns per group (480)
NCH = D // 128              # 128-partition chunks of D (8)
AF = mybir.ActivationFunctionType

# winograd j-streams: widths per (j, ck) block of the rhs / psum planes.
# j0,j1,j2,j5 carry [A(48) | B(48) | C(48)]; j3,j4 carry [A | B].
JW = [144, 144, 144, 96, 96, 144]
JCUM = [0, 144, 288, 432, 528, 624]
CKW = 768                   # total cols per ck block (= sum(JW))
# psum plane placement: j -> (bank, col offset) inside a [128, 2, 512] tile;
# j-accumulation groups sharing a bank run strictly sequentially
# (bank0: j0, j1, j3; bank1: j2, j4, j5 in program order j0..j5).
PL = {0: (0, 0), 1: (0, 144), 3: (0, 288), 2: (1, 0), 5: (1, 144), 4: (1, 288)}

F43_PTS = [0.0, 1.0, -1.0, 2.0, -0.5]
F23_PTS = [0.0, 1.0, -1.0]


def _cook_toom(points, m, r):
    """Winograd F(m, r) matrices: y = AT @ ((G @ w) * (BT @ x))."""
    a = np.asarray(points, np.float64)
    n = m + r - 1
    AT = np.zeros((m, n))
    for j in range(n - 1):
        AT[:, j] = a[j] ** np.arange(m)
    AT[m - 1, n - 1] = 1.0
    G = np.zeros((n, r))
    for j in range(n - 1):
        fj = np.prod(np.delete(a[j] - a, j))
        G[j] = (a[j] ** np.arange(r)) / fj
    G[n - 1, r - 1] = 1.0
    BT = np.zeros((n, n))
    for j in range(n - 1):
        BT[j, :n - 1] = np.poly(np.delete(a, j))[::-1]
    BT[n - 1] = np.poly(a)[::-1]
    return AT, G, BT


AT6, G6, BT6 = _cook_toom(F43_PTS, 4, 3)
AT4, G4, BT4 = _cook_toom(F23_PTS, 2, 3)
# F23 weights are scalar multiples of F43 ones at the shared points
# (j 0,1,2 <-> c 0,1,2) and identical at infinity (j5 <-> c3).
_CJ = [0, 1, 2, 5]          # F43 j-index serving F23 column c
_CRATIO = np.array([
    np.dot(G6[j], G4[c]) / np.dot(G4[c], G4[c])
    for c, j in enumerate(_CJ)
])
# fold the m4 plane scale so its A^T coefficients become (-8, 4, -2, 1)
M4SCALE = -0.125


def _build():
    nc = bacc.Bacc("TRN2", target_bir_lowering=False, debug=False)

    # xt: [group, cc, (j, ck, cols)] winograd-transformed inputs
    xt = nc.dram_tensor("xt", [NG, 128, NCH * CKW], mybir.dt.bfloat16,
                        kind="ExternalInput")
    # cwr[dck]: [cc, ((j*NCH+ck)*128 + dd)] winograd conv lhsT blocks
    cwr = nc.dram_tensor("cwr", [NCH, 128, 6 * NCH * 128], mybir.dt.bfloat16,
                         kind="ExternalInput")
    # gwr: [dd, ((eck*NCH+dck)*128 + ee)] gate lhsT blocks (single tensor)
    gwr = nc.dram_tensor("gwr", [128, NCH * NCH * 128], mybir.dt.bfloat16,
                         kind="ExternalInput")
    cbg = nc.dram_tensor("cbg", [128, 2 * NCH], mybir.dt.float32,
                         kind="ExternalInput")
    outT = nc.dram_tensor("outT", [D, RC], mybir.dt.bfloat16,
                          kind="ExternalOutput")

    with tile.TileContext(nc) as tc:
        with (
            tc.tile_pool(name="consts", bufs=1) as consts,
            tc.tile_pool(name="work", bufs=2) as work,
            tc.tile_pool(name="cnn", bufs=2) as cnnp,
            tc.tile_pool(name="cpsum", bufs=3, space="PSUM") as cpsum,
            tc.tile_pool(name="gpsum", bufs=2, space="PSUM") as gpsum,
        ):
            cbg_sb = consts.tile([128, 2 * NCH], mybir.dt.float32, tag="cbg")
            cb_sb = cbg_sb[:, :NCH]
            gb_sb = cbg_sb[:, NCH:]

            xt_sb = [None] * NG
            cwr_sb = [None] * NCH

            def alloc_xg(g):
                xt_sb[g] = consts.tile([128, NCH * CKW], mybir.dt.bfloat16,
                                       name=f"xg{g}", tag=f"xg{g}")

            def alloc_cw(dck):
                cwr_sb[dck] = consts.tile([128, 6 * NCH * 128],
                                          mybir.dt.bfloat16,
                                          name=f"cw{dck}", tag=f"cw{dck}")

            def load_xg_jp(g, j0, j1):
                lo, hi = NCH * JCUM[j0], NCH * (JCUM[j1] + JW[j1])
                nc.sync.dma_start(xt_sb[g][:, lo:hi], xt[g][:, lo:hi])

            def load_cw_jp(dck, j0, j1):
                lo, hi = j0 * NCH * 128, (j1 + 1) * NCH * 128
                nc.sync.dma_start(cwr_sb[dck][:, lo:hi], cwr[dck][:, lo:hi])

            # Input stream in first-use order on the single Sync HWDGE
            # queue. Each dma_start costs ~650 ns of Sync issue time, so
            # only the data the first conv j-streams block on is sliced
            # (in j-pairs); everything later ships whole. xt2/xt3 precede
            # the gate weights: they are needed at the pair-1 convs, the
            # gate weights only at the first interleaved gate quad.
            for g in range(NG):
                alloc_xg(g)
            for dck in range(NCH):
                alloc_cw(dck)
            for j0 in (0, 2, 4):
                load_xg_jp(0, j0, j0 + 1)
                load_xg_jp(1, j0, j0 + 1)
                load_cw_jp(0, j0, j0 + 1)
                if j0 == 0:
                    nc.sync.dma_start(cbg_sb[:], cbg[:])
            for dck in range(1, NCH):
                if dck <= 3:
                    load_cw_jp(dck, 0, 1)
                    load_cw_jp(dck, 2, 5)
                else:
                    nc.sync.dma_start(cwr_sb[dck][:], cwr[dck])
            gw_sb = consts.tile([128, NCH * NCH * 128], mybir.dt.bfloat16,
                                tag="gw")
            half = NCH * NCH * 128 // 2
            nc.sync.dma_start(gw_sb[:, :half], gwr[:, :half])
            nc.sync.dma_start(xt_sb[2][:], xt[2])
            nc.sync.dma_start(xt_sb[3][:], xt[3])
            nc.sync.dma_start(gw_sb[:, half:], gwr[:, half:])

            # Warm-up during the input-DMA bubble: throwaway matmuls flip
            # the PE HAM clock gate up just as the real stream starts.
            scr = consts.tile([128, 512], mybir.dt.bfloat16, tag="scr")
            nc.vector.memset(scr[:], 0.0)
            wps = gpsum.tile([128, GN], mybir.dt.float32, tag="gps")
            for _ in range(12):
                nc.tensor.matmul(wps[:, :480], scr[:, :128], scr[:, :480],
                                 start=True, stop=True)

            def conv_pair(ga, gb_, dck):
                psA = cpsum.tile([128, 2, 512], mybir.dt.float32, tag="cps")
                psB = cpsum.tile([128, 2, 512], mybir.dt.float32, tag="cps")
                for j in range(6):
                    bk, off = PL[j]
                    wj = JW[j]
                    for ck in range(NCH):
                        rlo = NCH * JCUM[j] + ck * wj
                        for g, ps in ((ga, psA), (gb_, psB)):
                            nc.tensor.matmul(
                                ps[:, bk, off:off + wj],
                                cwr_sb[dck][:, (j * NCH + ck) * 128:
                                            (j * NCH + ck + 1) * 128],
                                xt_sb[g][:, rlo:rlo + wj],
                                start=(ck == 0),
                                stop=(ck == NCH - 1),
                            )
                return psA, psB

            def epilogue(ps, dck, ptag):
                """A^T combine: m planes -> cnn tile [128, GN] bf16,
                columns ordered (t, win)."""
                def AB(j):
                    bk, off = PL[j]
                    return ps[:, bk, off:off + 96]

                def CC(j):
                    bk, off = PL[j]
                    return ps[:, bk, off + 96:off + 144]

                cbs = cb_sb[:, dck:dck + 1]
                # ScalarE: power-of-2 scaled copies of the m3 / m4 planes
                m3_2 = work.tile([128, 96], mybir.dt.bfloat16, tag="m3_2")
                nc.scalar.activation(m3_2[:], AB(3), AF.Copy, scale=2.0)
                m3_4 = work.tile([128, 96], mybir.dt.bfloat16, tag="m3_4")
                nc.scalar.activation(m3_4[:], m3_2[:], AF.Copy, scale=2.0)
                m3_8 = work.tile([128, 96], mybir.dt.bfloat16, tag="m3_8")
                nc.scalar.activation(m3_8[:], m3_4[:], AF.Copy, scale=2.0)
                # m4 plane is host-scaled by -1/8: coefficients (-8, 4, -2, 1)
                m4n2 = work.tile([128, 96], mybir.dt.bfloat16, tag="m4n2")
                nc.scalar.activation(m4n2[:], AB(4), AF.Copy, scale=-2.0)
                m4_4 = work.tile([128, 96], mybir.dt.bfloat16, tag="m4_4")
                nc.scalar.activation(m4_4[:], m4n2[:], AF.Copy, scale=-2.0)
                m4n8 = work.tile([128, 96], mybir.dt.bfloat16, tag="m4n8")
                nc.scalar.activation(m4n8[:], m4_4[:], AF.Copy, scale=-2.0)
                # ScalarE also stages the C-tile planes the GpSimd chain
                # needs (GpSimd cannot read PSUM).
                mc0s = work.tile([128, 48], mybir.dt.bfloat16, tag="mc0s")
                nc.scalar.activation(mc0s[:], CC(0), AF.Copy)
                mc2s = work.tile([128, 48], mybir.dt.bfloat16, tag="mc2s")
                nc.scalar.activation(mc2s[:], CC(2), AF.Copy)
                mc5s = work.tile([128, 48], mybir.dt.bfloat16, tag="mc5s")
                nc.scalar.activation(mc5s[:], CC(5), AF.Copy)
                # VectorE: chains. conv bias rides on the whole j1 plane (its
                # A^T column is all-ones for both the F(4,3) and F(2,3)
                # parts) so every output picks it up in one op.
                bk1, off1 = PL[1]
                m1b = work.tile([128, 144], mybir.dt.bfloat16, tag="m1b")
                nc.vector.tensor_scalar_add(m1b[:], ps[:, bk1, off1:off1 + 144],
                                            cbs)
                s = work.tile([128, 96], mybir.dt.bfloat16, tag="s")
                nc.vector.tensor_add(s[:], m1b[:, :96], AB(2))
                d_ = work.tile([128, 96], mybir.dt.bfloat16, tag="d_")
                nc.vector.tensor_sub(d_[:], m1b[:, :96], AB(2))

                ct = cnnp.tile([128, GN], mybir.dt.bfloat16,
                               tag=f"cnn{ptag}_{dck}")
                ctv = ct[:].rearrange("q (t w) -> q t w", t=W)

                def pw(t):
                    return ctv[:, t:t + 5:4]

                t1 = work.tile([128, 96], mybir.dt.bfloat16, tag="t1")
                nc.vector.tensor_add(t1[:], AB(0), s[:])
                t2 = work.tile([128, 96], mybir.dt.bfloat16, tag="t2")
                nc.vector.tensor_add(t2[:], t1[:], AB(3))
                nc.vector.tensor_add(pw(0), t2[:], m4n8[:])
                t3 = work.tile([128, 96], mybir.dt.bfloat16, tag="t3")
                nc.vector.tensor_add(t3[:], d_[:], m3_2[:])
                nc.vector.tensor_add(pw(1), t3[:], m4_4[:])
                t4 = work.tile([128, 96], mybir.dt.bfloat16, tag="t4")
                nc.vector.tensor_add(t4[:], s[:], m3_4[:])
                nc.vector.tensor_add(pw(2), t4[:], m4n2[:])
                t5 = work.tile([128, 96], mybir.dt.bfloat16, tag="t5")
                nc.vector.tensor_add(t5[:], d_[:], AB(5))
                t6 = work.tile([128, 96], mybir.dt.bfloat16, tag="t6")
                nc.vector.tensor_add(t6[:], t5[:], m3_8[:])
                nc.vector.tensor_add(pw(3), t6[:], AB(4))
                # C tile (F(2,3)) on GpSimd (SBUF-only inputs):
                # y8 = mc0+mc1+mc2, y9 = mc1-mc2+mc3; mc1+cb is m1b's tail.
                mc1b = m1b[:, 96:144]
                t7 = work.tile([128, 48], mybir.dt.bfloat16, tag="t7")
                nc.gpsimd.tensor_add(t7[:], mc1b, mc0s[:])
                nc.gpsimd.tensor_add(ctv[:, 8], t7[:], mc2s[:])
                t8 = work.tile([128, 48], mybir.dt.bfloat16, tag="t8")
                nc.gpsimd.tensor_sub(t8[:], mc1b, mc2s[:])
                nc.gpsimd.tensor_add(ctv[:, 9], t8[:], mc5s[:])
                return ct

            outv = outT[:].rearrange("(e r) c -> r e c", r=128)

            def gate_quad(g, eh, cnnT, lasth=False):
                # gateT[e, r] = sigmoid(sum_d gw[d, e] * cnnT[d, r] + gb[e])
                # for 4 eck chunks; one batched output DMA per quad.
                ot4 = work.tile([128, 4, GN], mybir.dt.bfloat16, tag="ot4",
                                bufs=2)
                for ei in range(4):
                    eck = eh * 4 + ei
                    ps2 = gpsum.tile([128, GN], mybir.dt.float32, tag="gps")
                    for dck in range(NCH):
                        nc.tensor.matmul(
                            ps2[:],
                            gw_sb[:, (eck * NCH + dck) * 128:
                                  (eck * NCH + dck + 1) * 128],
                            cnnT[dck][:],
                            start=(dck == 0),
                            stop=(dck == NCH - 1),
                        )
                    gt = work.tile([128, GN], mybir.dt.bfloat16, tag="gate",
                                   bufs=2)
                    chunks = ((0, 120), (120, 240), (240, 360), (360, GN)) if (
                        lasth and ei == 3) else ((0, GN),)
                    for lo, hi in chunks:
                        nc.scalar.activation(gt[:, lo:hi], ps2[:, lo:hi],
                                             AF.Sigmoid,
                                             bias=gb_sb[:, eck:eck + 1])
                        nc.vector.tensor_mul(ot4[:, ei, lo:hi],
                                             cnnT[eck][:, lo:hi],
                                             gt[:, lo:hi])
                        if lasth:
                            nc.sync.dma_start(
                                outv[:, eck:eck + 1,
                                     g * GN + lo:g * GN + hi],
                                ot4[:, ei:ei + 1, lo:hi])
                if not lasth:
                    nc.sync.dma_start(
                        outv[:, eh * 4:(eh + 1) * 4, g * GN:(g + 1) * GN],
                        ot4[:])

            cnn_tiles = [[None] * NCH for _ in range(NG)]
            # pair 0 convs
            for dck in range(NCH):
                psA, psB = conv_pair(0, 1, dck)
                cnn_tiles[0][dck] = epilogue(psA, dck, 0)
                cnn_tiles[1][dck] = epilogue(psB, dck, 0)
            # pair 0 gates interleaved with pair 1 convs: the epilogue's
            # VectorE/ScalarE work spreads into the gate matmul stretches,
            # where those engines would otherwise idle.
            for dck in range(NCH):
                if dck % 2 == 0:
                    k = dck // 2
                    gate_quad(k // 2, k % 2, cnn_tiles[k // 2])
                psA, psB = conv_pair(2, 3, dck)
                cnn_tiles[2][dck] = epilogue(psA, dck, 1)
                cnn_tiles[3][dck] = epilogue(psB, dck, 1)
            # pair 1 gates
            for g in (2, 3):
                for eh in range(2):
                    gate_quad(g, eh, cnn_tiles[g],
                              lasth=(g == 3 and eh == 1))
    nc.compile()
    return nc


def _prep_core_input(x_shard, cw_host, gw_host, cbg_host):
    # x_shard: [BC, T, D] -> winograd-transformed [NG, 128, NCH*CKW]
    xs = x_shard.reshape(NG, GWIN, W, D)
    xp = np.zeros((NG, GWIN, 12, D), np.float32)
    xp[:, :, 1:1 + W, :] = xs
    dA = np.einsum('ji,gwic->gwjc', BT6.astype(np.float32), xp[:, :, 0:6],
                   optimize=True)
    dB = np.einsum('ji,gwic->gwjc', BT6.astype(np.float32), xp[:, :, 4:10],
                   optimize=True)
    dC = np.einsum('ji,gwic->gwjc', BT4.astype(np.float32), xp[:, :, 8:12],
                   optimize=True)
    # assemble [NG, D, (j, block)] then chunk D -> (ck, cc) with j-major
    # free layout [(j, ck, cols)]
    blk = np.empty((NG, D, CKW), np.float32)
    for j in range(6):
        o = JCUM[j]
        blk[:, :, o:o + 48] = dA[:, :, j].transpose(0, 2, 1)
        blk[:, :, o + 48:o + 96] = dB[:, :, j].transpose(0, 2, 1)
        if j in _CJ:
            c = _CJ.index(j)
            blk[:, :, o + 96:o + 144] = (np.float32(1.0 / _CRATIO[c])
                                         * dC[:, :, c]).transpose(0, 2, 1)
    # [NG, D, (j, cols48*w)] -> [NG, ck, cc, j, wj] j-major per ck? Need
    # layout [(j, ck, wj)]: currently blk is [NG, D, (j, wj)] — reorder to
    # j-major over ck: final free index = (j, ck, wj)
    xt_host = np.empty((NG, 128, NCH * CKW), np.float32)
    bv = blk.reshape(NG, NCH, 128, CKW)
    for j in range(6):
        o, wj = JCUM[j], JW[j]
        dst = xt_host[:, :, NCH * o:NCH * (o + wj)].reshape(
            NG, 128, NCH, wj)
        dst[:] = bv[:, :, :, o:o + wj].transpose(0, 2, 1, 3)
    return {"xt": xt_host.astype(BF16), "cwr": cw_host, "gwr": gw_host,
            "cbg": cbg_host}


def _prep_in_maps(x, conv_w, conv_b, gate_w, gate_b):
    # conv weight transform + lhsT blocks:
    # cwr[dck][cc, (j*NCH+ck)*128 + dd] = gw_j[dck*128+dd, ck*128+cc]
    gw6 = np.einsum('jk,dck->jdc', G6.astype(np.float32), conv_w,
                    optimize=True)
    gw6[4] *= np.float32(M4SCALE)
    gt = gw6.reshape(6, NCH, 128, NCH, 128)  # [j, dck, dd, ck, cc]
    cw_host = np.ascontiguousarray(gt.transpose(1, 4, 0, 3, 2)).reshape(
        NCH, 128, 6 * NCH * 128).astype(BF16)
    # gate lhsT blocks: gwr[dd, (eck*NCH+dck)*128 + ee]
    gwt = gate_w.T.reshape(NCH, 128, NCH, 128)  # [dck, dd, eck, ee]
    gw_host = np.ascontiguousarray(gwt.transpose(1, 2, 0, 3)).reshape(
        128, NCH * NCH * 128).astype(BF16)
    cbg_host = np.ascontiguousarray(np.concatenate(
        [conv_b.reshape(NCH, 128).T, gate_b.reshape(NCH, 128).T],
        axis=1)).astype(np.float32)
    return [
        _prep_core_input(x[BC * i:BC * (i + 1)], cw_host, gw_host, cbg_host)
        for i in range(NCORES)
    ]


def _unshard_core(o):
    # o: [D, RC] bf16 with columns ordered (group, t, win) -> [BC, T, D] f32
    return (np.asarray(o).astype(np.float32)
            .reshape(D, NG, W, GWIN).transpose(1, 3, 2, 0)
            .reshape(NWIN, W, D).reshape(BC, T, D))


_NC_CACHE = None


def kernel(x, conv_w, conv_b, gate_w, gate_b):
    global _NC_CACHE
    x = np.asarray(x, np.float32)
    conv_w = np.asarray(conv_w, np.float32)
    conv_b = np.asarray(conv_b, np.float32)
    gate_w = np.asarray(gate_w, np.float32)
    gate_b = np.asarray(gate_b, np.float32)

    in_maps = _prep_in_maps(x, conv_w, conv_b, gate_w, gate_b)
    if _NC_CACHE is None:
        _NC_CACHE = _build()
    res = run_bass_kernel_spmd(_NC_CACHE, in_maps, core_ids=list(range(NCORES))).results

    out = np.empty((B, T, D), np.float32)
    for i in range(NCORES):
        out[BC * i:BC * (i + 1)] = _unshard_core(res[i]["outT"])
    return out
